# revision 2
# baseline (speedup 1.0000x reference)
"""Trainium2 Bass kernel: GNN message-passing layer (nn_GNNlayer).

Computes, for full inputs (A [N,N], x [N,DIN], theta [K], W [DOUT,DIN], b, k):
    S1 = D^-1/2 A D^-1/2           (D = diag(rowsum A))
    P  = I + t0*S1 + t1*S1^2       (t = sigmoid(theta))
    S2 = D2^-1/2 P D2^-1/2         (D2 = diag(rowsum P))
    M  = top-k mask per row of S2
    out = (S2*M) @ x @ W.T + b

Sharding: rows are split across 8 NeuronCores (512 rows each). Each core
streams the full A as the moving matmul operand, computes its row block of
everything, and two tiny AllGathers exchange the degree vectors d and d2.

Per-core algebra (rows R), scaled by G=2048 to keep fp8 weights normal:
    B' = A_R @ diag(G*t1/d) @ A                (fp8 DoubleRow matmul)
    C' = (G*t0)*A_R + B'            = G * (t0*A_R + t1*S1^2-numerator)
    d2_i = 1 + dinv_i * (1/G) * sum_j C'_ij * dinv_j
    ranking value T_ij = C'_ij * dinv_j * dinv2b_j   (monotone in true S2)
    row top-k = {diagonal} + top-(k-1) of T (diag excluded via max8 filter)
    out_i = [c_off_i * sum_t T_t * x[j_t] + c_diag_i * x[i]] @ W.T + b
      c_off_i  = dinv2b_i * dinv_i / G
      c_diag_i = dinv2b_i^2 * (1 + dinv_i^2 * C'_ii / G)

The big matmul runs in fp8 e4m3 with DoubleRow perf mode (2 contraction
rows per PE pass). The accumulating C block and the ranking pipeline are
bf16: ranking noise only reorders near-tied off-diagonal picks whose S2
values differ by O(1e-6), far below the output tolerance.
"""

import os
import sys
import time
from contextlib import ExitStack

import numpy as np

sys.path.insert(0, "/opt/trn_rl_repo")

import concourse.bass as bass  # noqa: E402
import concourse.tile as tile  # noqa: E402
from concourse import bacc, bass_utils, mybir  # noqa: E402
from concourse.masks import make_identity  # noqa: E402

P = 128
NCORES = 8
G = 2048.0  # fp8 weight prescale; folded back out of the tiny row coefficients

f32 = mybir.dt.float32
f32r = mybir.dt.float32r
bf16 = mybir.dt.bfloat16
f8 = mybir.dt.float8e4
u32 = mybir.dt.uint32
i16 = mybir.dt.int16
OP = mybir.AluOpType
AF = mybir.ActivationFunctionType
AX = mybir.AxisListType
PM = mybir.MatmulPerfMode

BIGNEG = 1.0e30

LAST_RUN_INFO = {}
_PROGRAM_CACHE = {}


def _geom(N, NB):
    NT = NB // P              # 128-row tiles per block (4)
    PW = 1024                 # phase width (columns of A per phase)
    PHASES = N // PW          # 4
    NPAIR = N // (2 * P)      # DoubleRow contraction chunk-pairs (16)
    JT = PW // 256            # 256-col output subtiles per phase (4)
    LC = N // P               # 128-row contraction chunks (32), for wscale
    return NT, PW, PHASES, NPAIR, JT, LC


def _emit(tc, io, N, NB, DIN, k):
    STAGE = int(os.environ.get("K_STAGE", "9"))
    nc = tc.nc
    topn = k - 1              # off-diagonal picks per row
    NT, PW, PHASES, NPAIR, JT, LC = _geom(N, NB)

    ctx = ExitStack()
    with ctx:
        cst = ctx.enter_context(tc.tile_pool(name="cst", bufs=1))
        cpool = ctx.enter_context(tc.tile_pool(name="cmat", bufs=1))
        wpool = ctx.enter_context(tc.tile_pool(name="wmat", bufs=1))
        dram = ctx.enter_context(tc.tile_pool(name="dram", bufs=1, space="DRAM"))

        # ---------------- constants / tiny loads ----------------
        ident = cst.tile([P, P], f32)
        make_identity(nc, ident[:])

        col_iota = cst.tile([P, N], i16)
        nc.gpsimd.iota(col_iota[:], pattern=[[1, N]], base=0, channel_multiplier=0)

        iota8 = cst.tile([P, 8], i16)
        nc.gpsimd.iota(iota8[:], pattern=[[1, 8]], base=0, channel_multiplier=0)
        iota8f = cst.tile([P, 8], f32)
        nc.vector.tensor_copy(out=iota8f[:], in_=iota8[:])
        iota8m = cst.tile([P, 8], f32)  # t - 99
        nc.vector.tensor_scalar_add(iota8m[:], iota8f[:], -99.0)

        th_b = cst.tile([P, 2], f32)
        nc.sync.dma_start(out=th_b[:], in_=io["theta"].broadcast_to([P, 2]))
        # sigmoid(theta) = 1 / (1 + exp(-theta)); DVE reciprocal for accuracy
        th_e = cst.tile([P, 2], f32)
        nc.scalar.activation(th_e[:], th_b[:], AF.Exp, scale=-1.0)
        nc.vector.tensor_scalar_add(th_e[:], th_e[:], 1.0)
        ts_sb = cst.tile([P, 2], f32)
        nc.vector.reciprocal(ts_sb[:], th_e[:])
        tsg = cst.tile([P, 2], f32)  # sigmoid(theta) * G
        nc.vector.tensor_scalar_mul(tsg[:], ts_sb[:], G)

        b_rep = cst.tile([P, DIN], f32)
        nc.sync.dma_start(out=b_rep[:], in_=io["bvec"].broadcast_to([P, DIN]))

        wt_sb = cst.tile([DIN, DIN], f32r)
        nc.sync.dma_start(out=wt_sb[:], in_=io["wt"])

        rowf_sb = cst.tile([P, NT], f32)
        nc.sync.dma_start(out=rowf_sb[:], in_=io["rowf"])

        # raw fp8 weights (pair-interleaved A^T of the block): no deps, load now
        w8 = []
        for pr in range(NPAIR):
            wtile = wpool.tile([P, 2, NB], f8, tag=f"w{pr}", name=f"w{pr}")
            nc.sync.dma_start(
                out=wtile[:],
                in_=io["a_t8"][pr * P:(pr + 1) * P, :].rearrange(
                    "p (two b) -> p two b", two=2))
            w8.append(wtile)

        # ---------------- block load (bf16) + degree d ----------------
        C = [cpool.tile([P, N], bf16, tag=f"c{it}", name=f"C{it}") for it in range(NT)]
        dm_sb = cst.tile([P, NT], f32)
        dmq = cst.tile([P, 4], f32)
        for it in range(NT):
            quart = N // 4
            for qq in range(4):
                nc.sync.dma_start(
                    out=C[it][:, qq * quart:(qq + 1) * quart],
                    in_=io["a_blk"][it * P:(it + 1) * P, qq * quart:(qq + 1) * quart])
                nc.vector.tensor_reduce(out=dmq[:, qq:qq + 1],
                                        in_=C[it][:, qq * quart:(qq + 1) * quart],
                                        axis=AX.X, op=OP.add)
            nc.vector.tensor_reduce(out=dm_sb[:, it:it + 1], in_=dmq[:],
                                    axis=AX.X, op=OP.add)
        # local per-row scale factors
        dinv2_blk = cst.tile([P, NT], f32)
        nc.vector.reciprocal(dinv2_blk[:], dm_sb[:])
        dinv_blk = cst.tile([P, NT], f32)
        nc.scalar.activation(dinv_blk[:], dinv2_blk[:], AF.Sqrt)
        if STAGE < 2:
            return
        # ---------------- AllGather #1 (d) ----------------
        g1_in = dram.tile([NT, P], f32)
        g1_out = dram.tile([LC, P], f32)
        with tc.tile_pool(name="psA", bufs=2, space="PSUM") as psA:
            dmT_ps = psA.tile([NT, P], f32)
            nc.tensor.transpose(out=dmT_ps[:], in_=dm_sb[:], identity=ident[:])
            dmT = cst.tile([NT, P], f32)
            nc.scalar.activation(dmT[:], dmT_ps[:], AF.Copy)
            nc.sync.dma_start(out=g1_in[:], in_=dmT[:])
            nc.gpsimd.collective_compute(
                "AllGather", OP.bypass,
                replica_groups=[list(range(NCORES))],
                ins=[g1_in.opt()], outs=[g1_out.opt()],
            )
            # (G*t0)-scale of the block overlaps the AllGather wait
            for it in range(NT):
                nc.vector.tensor_scalar_mul(C[it][:], C[it][:], tsg[:, 0:1])
            da_sb = cst.tile([LC, P], f32)
            nc.sync.dma_start(out=da_sb[:], in_=g1_out[:])

            # dinv (flat, global order, bf16) to DRAM for broadcast loads
            rda = cst.tile([LC, P], f32)
            nc.vector.reciprocal(rda[:], da_sb[:])
            dinv32 = cst.tile([LC, P], f32)
            nc.scalar.activation(dinv32[:], rda[:], AF.Sqrt)
            dinvb = cst.tile([LC, P], bf16)
            nc.vector.tensor_copy(out=dinvb[:], in_=dinv32[:])
            dinv_flat = dram.tile([1, N], bf16)
            nc.sync.dma_start(
                out=dinv_flat[:].rearrange("one (a b) -> (one a) b", a=LC),
                in_=dinvb[:])

            # W column scale = G*t1 / d  in [P, LC] layout (transpose of rda)
            wsc_ps = psA.tile([P, LC], f32)
            nc.tensor.transpose(out=wsc_ps[:], in_=rda[:], identity=ident[:LC, :LC])
            wscale = cst.tile([P, LC], f32)
            nc.scalar.activation(wscale[:], wsc_ps[:], AF.Copy, scale=tsg[:, 1:2])

        if STAGE < 3:
            return
        # scale raw fp8 weights in place: slot i of pair pr scaled by G*t1/d[lc]
        for pr in range(NPAIR):
            for i in range(2):
                lc = 2 * pr + i
                nc.scalar.activation(w8[pr][:, i, :], w8[pr][:, i, :],
                                     AF.Copy, scale=wscale[:, lc:lc + 1])

        if STAGE < 4:
            return
        # ---------------- main loop: B' = A_R @ diag(G t1/d) @ A ----------------
        mainctx = ExitStack()
        with mainctx:
            astream = mainctx.enter_context(tc.tile_pool(name="astream", bufs=12))
            dsp = mainctx.enter_context(tc.tile_pool(name="dstripe", bufs=2))
            scrp = mainctx.enter_context(tc.tile_pool(name="scr", bufs=2))
            psM = mainctx.enter_context(tc.tile_pool(name="psM", bufs=8,
                                                     space="PSUM"))

            d2part = [cst.tile([P, PHASES], f32, tag=f"d2p{it}", name=f"d2p{it}")
                      for it in range(NT)]
            ciipart = [cst.tile([P, PHASES], f32, tag=f"cip{it}", name=f"cip{it}")
                       for it in range(NT)]

            for q in range(PHASES):
                q0 = q * PW
                dstripe = dsp.tile([P, PW], bf16, tag="dstripe")
                nc.sync.dma_start(out=dstripe[:],
                                  in_=dinv_flat[:, q0:q0 + PW].broadcast_to([P, PW]))
                psums = [psM.tile([P, 512], f32, tag="acc", name=f"ps{q}_{i}")
                         for i in range(2 * NT)]
                for pr in range(NPAIR):
                    asl = astream.tile([P, 2, PW], f8, tag="astream")
                    # pair-interleaved phase-major layout: each partition line
                    # is 2KB-contiguous; split by partition halves over queues
                    for rq in range(2):
                        nc.sync.dma_start(
                            out=asl[rq * 64:(rq + 1) * 64, :, :],
                            in_=io["a_ph8"][(q * NPAIR + pr) * P + rq * 64:
                                            (q * NPAIR + pr) * P + (rq + 1) * 64,
                                            :].rearrange("p (two w) -> p two w",
                                                         two=2))
                    for it in range(NT):
                        lhs = w8[pr][:, :, it * P:(it + 1) * P]
                        for jt in range(JT):
                            bank = it * 2 + jt // 2
                            half = jt % 2
                            nc.tensor.matmul(
                                out=psums[bank][:, half * 256:(half + 1) * 256],
                                lhsT=lhs,
                                rhs=asl[:, :, jt * 256:(jt + 1) * 256],
                                start=(pr == 0), stop=(pr == NPAIR - 1),
                                perf_mode=PM.DoubleRow)
                # free all PSUM banks first (PE of phase q+1 waits on these),
                # then the d2 row-sum partials
                for it in range(NT):
                    for jc in range(2):
                        cs = C[it][:, q0 + jc * 512:q0 + (jc + 1) * 512]
                        nc.vector.tensor_add(out=cs, in0=psums[it * 2 + jc][:],
                                             in1=cs)
                for it in range(NT):
                    stripe = C[it][:, q0:q0 + PW]
                    scr1 = scrp.tile([P, PW], bf16, tag="scr")
                    nc.vector.tensor_mul(scr1[:], stripe, dstripe[:])
                    nc.vector.tensor_reduce(out=d2part[it][:, q:q + 1],
                                            in_=scr1[:], axis=AX.X, op=OP.add)
                    scr2 = scrp.tile([P, PW], bf16, tag="scr")
                    nc.vector.tensor_scalar(out=scr2[:],
                                            in0=col_iota[:, q0:q0 + PW],
                                            scalar1=rowf_sb[:, it:it + 1],
                                            scalar2=None, op0=OP.is_equal)
                    nc.vector.tensor_mul(scr2[:], scr2[:], stripe)
                    nc.vector.tensor_reduce(out=ciipart[it][:, q:q + 1],
                                            in_=scr2[:], axis=AX.X, op=OP.add)

        if STAGE < 5:
            return
        # ---------------- d2, AllGather #2, ranking scale ----------------
        tailctx = ExitStack()
        with tailctx:
            tp = tailctx.enter_context(tc.tile_pool(name="tail", bufs=1))
            tscr = tailctx.enter_context(tc.tile_pool(name="tscr", bufs=2))
            psT = tailctx.enter_context(tc.tile_pool(name="psT", bufs=4,
                                                     space="PSUM"))

            d2m = tp.tile([P, NT], f32)
            for it in range(NT):
                nc.vector.tensor_reduce(out=d2m[:, it:it + 1], in_=d2part[it][:],
                                        axis=AX.X, op=OP.add)
            # d2 = 1 + dinv * sum / G
            nc.vector.tensor_mul(d2m[:], d2m[:], dinv_blk[:])
            nc.vector.tensor_scalar(out=d2m[:], in0=d2m[:], scalar1=1.0 / G,
                                    scalar2=1.0, op0=OP.mult, op1=OP.add)

            dinv2b2 = tp.tile([P, NT], f32)  # dinv2b^2 = 1/d2
            nc.vector.reciprocal(dinv2b2[:], d2m[:])
            dinv2b = tp.tile([P, NT], f32)
            nc.scalar.activation(dinv2b[:], dinv2b2[:], AF.Sqrt)

            g2_in = dram.tile([NT, P], f32)
            g2_out = dram.tile([LC, P], f32)
            d2T_ps = psT.tile([NT, P], f32, tag="tr")
            nc.tensor.transpose(out=d2T_ps[:], in_=d2m[:], identity=ident[:])
            d2T = tp.tile([NT, P], f32)
            nc.scalar.activation(d2T[:], d2T_ps[:], AF.Copy)
            nc.sync.dma_start(out=g2_in[:], in_=d2T[:])
            nc.gpsimd.collective_compute(
                "AllGather", OP.bypass,
                replica_groups=[list(range(NCORES))],
                ins=[g2_in.opt()], outs=[g2_out.opt()],
            )

            cii = tp.tile([P, NT], f32)
            for it in range(NT):
                nc.vector.tensor_reduce(out=cii[:, it:it + 1], in_=ciipart[it][:],
                                        axis=AX.X, op=OP.add)

            d2a_sb = tp.tile([LC, P], f32)
            nc.sync.dma_start(out=d2a_sb[:], in_=g2_out[:])

            # v = 1/sqrt(d * d2)  (global order), replicate to all partitions
            vtmp = tp.tile([LC, P], f32)
            nc.vector.tensor_mul(vtmp[:], da_sb[:], d2a_sb[:])
            nc.vector.reciprocal(vtmp[:], vtmp[:])
            v32 = tp.tile([LC, P], f32)
            nc.scalar.activation(v32[:], vtmp[:], AF.Sqrt)
            vb = tp.tile([LC, P], bf16)
            nc.vector.tensor_copy(out=vb[:], in_=v32[:])
            v_flat = dram.tile([1, N], bf16)
            nc.sync.dma_start(
                out=v_flat[:].rearrange("one (a b) -> (one a) b", a=LC),
                in_=vb[:])
            v_rep = tp.tile([P, N], bf16)
            nc.sync.dma_start(out=v_rep[:], in_=v_flat[:].broadcast_to([P, N]))

            # output coefficients
            c_off = tp.tile([P, NT], f32)
            nc.vector.tensor_mul(c_off[:], dinv2b[:], dinv_blk[:])
            nc.vector.tensor_scalar_mul(c_off[:], c_off[:], 1.0 / G)
            c_diag = tp.tile([P, NT], f32)
            nc.vector.tensor_mul(c_diag[:], dinv2_blk[:], cii[:])
            nc.vector.tensor_scalar(out=c_diag[:], in0=c_diag[:], scalar1=1.0 / G,
                                    scalar2=1.0, op0=OP.mult, op1=OP.add)
            nc.vector.tensor_mul(c_diag[:], c_diag[:], dinv2b2[:])

            if STAGE < 6:
                return
            # ---------------- rank, gather, combine ----------------
            # per-tile max8/max_index on bf16 T; small ops [128,NT,8] in f32
            m8b = tp.tile([P, NT, 8], bf16)
            i8all = tp.tile([P, NT, 8], u32)
            for it in range(NT):
                T = tscr.tile([P, N], bf16, tag="dsel")
                nc.vector.tensor_mul(T[:], C[it][:], v_rep[:])
                nc.vector.max(out=m8b[:, it, :], in_=T[:])
                nc.vector.max_index(out=i8all[:, it, :], in_max=m8b[:, it, :],
                                    in_values=T[:])
            m8all = tp.tile([P, NT, 8], f32)
            nc.vector.tensor_copy(out=m8all[:], in_=m8b[:])
            i8f = tp.tile([P, NT, 8], f32)
            nc.vector.tensor_copy(out=i8f[:], in_=i8all[:])
            # rowf replicated along the 8-wide groups
            kill = tp.tile([P, NT, 8], f32)
            nc.vector.tensor_tensor(out=kill[:], in0=i8f[:],
                                    in1=rowf_sb[:].unsqueeze(2).to_broadcast([P, NT, 8]),
                                    op=OP.is_equal)
            # delete diag entry from candidates
            m8k = tp.tile([P, NT, 8], f32)
            nc.vector.tensor_scalar(out=m8k[:], in0=kill[:], scalar1=BIGNEG,
                                    scalar2=None, op0=OP.mult)
            nc.vector.tensor_sub(m8k[:], m8all[:], m8k[:])
            # diag position within each group (99 if absent)
            posw = tp.tile([P, NT, 8], f32)
            nc.vector.tensor_tensor(out=posw[:], in0=kill[:],
                                    in1=iota8m[:].unsqueeze(1).to_broadcast([P, NT, 8]),
                                    op=OP.mult)
            pos = tp.tile([P, NT], f32)
            nc.vector.tensor_reduce(out=pos[:], in_=posw[:], axis=AX.X, op=OP.add)
            nc.vector.tensor_scalar_add(pos[:], pos[:], 99.0)
            shift = tp.tile([P, NT, 8], f32)
            nc.vector.tensor_tensor(out=shift[:],
                                    in0=iota8f[:].unsqueeze(1).to_broadcast([P, NT, 8]),
                                    in1=pos[:].unsqueeze(2).to_broadcast([P, NT, 8]),
                                    op=OP.is_ge)
            # top-(k-1) values/indices skipping the diag slot
            val = tp.tile([P, NT, topn], f32)
            nc.vector.tensor_sub(val[:], m8k[:, :, 1:1 + topn], m8k[:, :, 0:topn])
            nc.vector.tensor_mul(val[:], val[:], shift[:, :, 0:topn])
            nc.vector.tensor_add(val[:], val[:], m8k[:, :, 0:topn])
            idxf = tp.tile([P, NT, topn], f32)
            nc.vector.tensor_sub(idxf[:], i8f[:, :, 1:1 + topn], i8f[:, :, 0:topn])
            nc.vector.tensor_mul(idxf[:], idxf[:], shift[:, :, 0:topn])
            nc.vector.tensor_add(idxf[:], idxf[:], i8f[:, :, 0:topn])
            idxu = tp.tile([P, NT, topn], u32)
            nc.vector.tensor_copy(out=idxu[:], in_=idxf[:])
            # coefficients applied to the gathered values
            cval = tp.tile([P, NT, topn], f32)
            nc.vector.tensor_tensor(out=cval[:], in0=val[:],
                                    in1=c_off[:].unsqueeze(2).to_broadcast([P, NT, topn]),
                                    op=OP.mult)

            # gathers (x rows): one batched indirect DMA per tile + one
            # strided direct DMA for the diagonal rows (own block of x)
            xg = tp.tile([P, NT, (topn + 1), DIN], f32)
            for it in range(NT):
                nc.gpsimd.indirect_dma_start(
                    out=xg[:, it, 0:topn, :], out_offset=None,
                    in_=io["x"],
                    in_offset=bass.IndirectOffsetOnAxis(ap=idxu[:, it, :],
                                                        axis=0))
            nc.sync.dma_start(
                out=xg[:, :, topn, :],
                in_=io["xblk"].rearrange("(nt pp) d -> pp nt d", nt=NT))

            # z = c_diag * x[i] + sum_t cval_t * x[j_t]   (batched over tiles)
            zall = tp.tile([P, NT, DIN], f32)
            nc.vector.tensor_tensor(out=zall[:], in0=xg[:, :, topn, :],
                                    in1=c_diag[:].unsqueeze(2).to_broadcast([P, NT, DIN]),
                                    op=OP.mult)
            zt = tp.tile([P, NT, DIN], f32)
            for t in range(topn):
                nc.vector.tensor_tensor(out=zt[:], in0=xg[:, :, t, :],
                                        in1=cval[:, :, t:t + 1].to_broadcast([P, NT, DIN]),
                                        op=OP.mult)
                nc.vector.tensor_add(zall[:], zall[:], zt[:])

            # out = z @ W.T + b  (per tile on PE)
            for it in range(NT):
                zT_ps = psT.tile([DIN, P], f32, tag="tr")
                nc.tensor.transpose(out=zT_ps[:], in_=zall[:, it, :],
                                    identity=ident[:])
                zT = tscr.tile([DIN, P], f32, tag="zT")
                nc.scalar.activation(zT[:].bitcast(f32r), zT_ps[:], AF.Copy)
                o_ps = psT.tile([P, DIN], f32, tag="ops")
                nc.tensor.matmul(out=o_ps[:], lhsT=zT[:].bitcast(f32r),
                                 rhs=wt_sb[:], start=True, stop=True)
                o_sb = tscr.tile([P, DIN], f32, tag="osb")
                nc.vector.tensor_add(o_sb[:], o_ps[:], b_rep[:])
                nc.sync.dma_start(out=io["out_blk"][it * P:(it + 1) * P, :],
                                  in_=o_sb[:])


def _build(N, NB, DIN, k):
    key = (N, NB, DIN, k, os.environ.get("K_STAGE", "9"))
    if key in _PROGRAM_CACHE:
        return _PROGRAM_CACHE[key]
    NT, PW, PHASES, NPAIR, JT, LC = _geom(N, NB)
    nc = bacc.Bacc("TRN2", target_bir_lowering=False, debug=False,
                   num_devices=NCORES)
    io = {
        "a_ph8": nc.dram_tensor("a_ph8", [PHASES * NPAIR * P, 2 * PW], f8,
                                kind="ExternalInput").ap(),
        "a_blk": nc.dram_tensor("a_blk", [NB, N], bf16,
                                kind="ExternalInput").ap(),
        "a_t8": nc.dram_tensor("a_t8", [NPAIR * P, 2 * NB], f8,
                               kind="ExternalInput").ap(),
        "x": nc.dram_tensor("x", [N, DIN], f32, kind="ExternalInput").ap(),
        "xblk": nc.dram_tensor("xblk", [NB, DIN], f32,
                               kind="ExternalInput").ap(),
        "wt": nc.dram_tensor("wt", [DIN, DIN], f32r, kind="ExternalInput").ap(),
        "bvec": nc.dram_tensor("bvec", [1, DIN], f32, kind="ExternalInput").ap(),
        "theta": nc.dram_tensor("theta", [1, 2], f32, kind="ExternalInput").ap(),
        "rowf": nc.dram_tensor("rowf", [P, NT], f32, kind="ExternalInput").ap(),
        "out_blk": nc.dram_tensor("out_blk", [NB, DIN], f32,
                                  kind="ExternalOutput").ap(),
    }
    with tile.TileContext(nc) as tc:
        _emit(tc, io, N, NB, DIN, k)
    nc.compile()
    _PROGRAM_CACHE[key] = nc
    return nc


def make_in_maps(x, A, theta, W, b, k, N, NB, DIN):
    A = np.ascontiguousarray(np.asarray(A, np.float32))
    x = np.ascontiguousarray(np.asarray(x, np.float32))
    theta = np.ascontiguousarray(np.asarray(theta, np.float32)).reshape(1, 2)
    W = np.asarray(W, np.float32)
    b = np.ascontiguousarray(np.asarray(b, np.float32)).reshape(1, DIN)
    wt = np.ascontiguousarray(W.T)
    NT, PW, PHASES, NPAIR, JT, LC = _geom(N, NB)
    F8 = mybir.dt.np(mybir.dt.float8e4)
    BF = mybir.dt.np(mybir.dt.bfloat16)
    A8 = A.astype(F8)
    # stream layout: row ((q*NPAIR + pair)*128 + p) holds A rows
    # (pair*256 + p) and (pair*256 + 128 + p), columns q*PW..(q+1)*PW
    a_ph8 = np.ascontiguousarray(
        A8.reshape(NPAIR, 2, P, PHASES, PW)
          .transpose(3, 0, 2, 1, 4)
          .reshape(PHASES * NPAIR * P, 2 * PW))
    in_maps = []
    for m in range(NCORES):
        rows = slice(m * NB, (m + 1) * NB)
        a_blk = np.ascontiguousarray(A[rows].astype(BF))
        # weights: row (pair*128 + p) = [A^T[pair*256+p, blk], A^T[pair*256+128+p, blk]]
        a_t8 = np.ascontiguousarray(
            A8[rows].T.reshape(NPAIR, 2, P, NB)
                      .transpose(0, 2, 1, 3)
                      .reshape(NPAIR * P, 2 * NB))
        ridx = (m * NB + np.arange(NB)).reshape(NT, P).T  # [P, NT]
        in_maps.append({
            "a_ph8": a_ph8,
            "a_blk": a_blk,
            "a_t8": a_t8,
            "x": x,
            "xblk": np.ascontiguousarray(x[rows]),
            "wt": wt,
            "bvec": b,
            "theta": theta,
            "rowf": np.ascontiguousarray(ridx.astype(np.float32)),
        })
    return in_maps


def kernel(x, A, theta, W, b, k, **extra):
    k = int(k)
    assert 1 <= k <= 8, f"k={k} unsupported"
    N = int(A.shape[0])
    DIN = int(x.shape[1])
    NB = N // NCORES
    nc = _build(N, NB, DIN, k)
    in_maps = make_in_maps(x, A, theta, W, b, k, N, NB, DIN)
    trace = bool(int(os.environ.get("BASS_KERNEL_TRACE", "0")))
    t0 = time.monotonic()
    res = bass_utils.run_bass_kernel_spmd(
        nc, in_maps, core_ids=list(range(NCORES)), trace=trace)
    t1 = time.monotonic()
    LAST_RUN_INFO.clear()
    LAST_RUN_INFO.update({
        "wall_s": t1 - t0,
        "exec_time_ns": res.exec_time_ns,
        "profile_json": res.profile_json,
    })
    out = np.concatenate([res.results[m]["out_blk"] for m in range(NCORES)], axis=0)
    return out.astype(np.float32)


# revision 8
# speedup vs baseline: 1.1437x; 1.1437x over previous
"""Trainium2 Bass kernel: GNN message-passing layer (nn_GNNlayer).

Computes, for full inputs (A [N,N], x [N,DIN], theta [K], W [DOUT,DIN], b, k):
    S1 = D^-1/2 A D^-1/2           (D = diag(rowsum A))
    P  = I + t0*S1 + t1*S1^2       (t = sigmoid(theta))
    S2 = D2^-1/2 P D2^-1/2         (D2 = diag(rowsum P))
    M  = top-k mask per row of S2
    out = (S2*M) @ x @ W.T + b

Sharding: rows are split across 8 NeuronCores (512 rows each). Each core
streams the full A as the moving matmul operand, computes its row block of
everything, and two tiny AllGathers exchange the degree vectors d and d2.

Per-core algebra (rows R), scaled by G=2048 to keep fp8 weights normal:
    B' = A_R @ diag(G*t1/d) @ A                (fp8 DoubleRow matmul)
    C' = (G*t0)*A_R + B'            = G * (t0*A_R + t1*S1^2-numerator)
    d2_i = 1 + dinv_i * (1/G) * sum_j C'_ij * dinv_j
    proxy rank U_ij = C'_ij * dinv_j; true T_ij = U_ij * dinv2b_j
    row top-k = {diagonal} + top-(k-1) of U (diag excluded via max8 filter);
      U-order vs T-order differ only among near-ties (dinv2b varies ~1%),
      which is far below the output tolerance.
    out_i = [c_off_i * sum_t U_t*dinv2b_{j_t} * x[j_t] + c_diag_i * x[i]] @ W.T + b
      c_off_i  = dinv2b_i * dinv_i / G
      c_diag_i = dinv2b_i^2 * (1 + dinv_i^2 * C'_ii / G)

The big matmul runs in fp8 e4m3 with DoubleRow perf mode (2 contraction
rows per PE pass). The accumulating C block is bf16. Ranking runs
per-phase on the d2 partial product (overlapped with the PE), so the tail
only merges 32 candidates per row tile.
"""

import os
import sys
import time
from contextlib import ExitStack

import numpy as np

sys.path.insert(0, "/opt/trn_rl_repo")

import concourse.bass as bass  # noqa: E402
import concourse.tile as tile  # noqa: E402
from concourse import bacc, bass_utils, mybir  # noqa: E402
from concourse.masks import make_identity  # noqa: E402

P = 128
NCORES = 8
G = 2048.0  # fp8 weight prescale; folded back out of the tiny row coefficients

f32 = mybir.dt.float32
f32r = mybir.dt.float32r
bf16 = mybir.dt.bfloat16
f8 = mybir.dt.float8e4
u32 = mybir.dt.uint32
i16 = mybir.dt.int16
OP = mybir.AluOpType
AF = mybir.ActivationFunctionType
AX = mybir.AxisListType
PM = mybir.MatmulPerfMode

BIGNEG = 1.0e30

LAST_RUN_INFO = {}
_PROGRAM_CACHE = {}


def _geom(N, NB):
    NT = NB // P              # 128-row tiles per block (4)
    PW = 1024                 # phase width (columns of A per phase)
    PHASES = N // PW          # 4
    NPAIR = N // (2 * P)      # DoubleRow contraction chunk-pairs (16)
    JT = PW // 256            # 256-col output subtiles per phase (4)
    LC = N // P               # 128-row contraction chunks (32), for wscale
    return NT, PW, PHASES, NPAIR, JT, LC


def _emit(tc, io, N, NB, DIN, k):
    STAGE = int(os.environ.get("K_STAGE", "9"))
    nc = tc.nc
    topn = k - 1              # off-diagonal picks per row
    NT, PW, PHASES, NPAIR, JT, LC = _geom(N, NB)
    NCAND = PHASES * 8        # merged ranking candidates per row tile (32)

    ctx = ExitStack()
    with ctx:
        cst = ctx.enter_context(tc.tile_pool(name="cst", bufs=1))
        cpool = ctx.enter_context(tc.tile_pool(name="cmat", bufs=1))
        wpool = ctx.enter_context(tc.tile_pool(name="wmat", bufs=1))
        dram = ctx.enter_context(tc.tile_pool(name="dram", bufs=1, space="DRAM"))

        # ---------------- constants / tiny loads ----------------
        ident = cst.tile([P, P], f32)
        make_identity(nc, ident[:])

        col_iota = cst.tile([P, N], i16)
        nc.gpsimd.iota(col_iota[:], pattern=[[1, N]], base=0, channel_multiplier=0)

        iota8 = cst.tile([P, 8], i16)
        nc.gpsimd.iota(iota8[:], pattern=[[1, 8]], base=0, channel_multiplier=0)
        iota8f = cst.tile([P, 8], f32)
        nc.vector.tensor_copy(out=iota8f[:], in_=iota8[:])
        iota8m = cst.tile([P, 8], f32)  # t - 99
        nc.vector.tensor_scalar_add(iota8m[:], iota8f[:], -99.0)

        iota32 = cst.tile([P, NCAND], i16)
        nc.gpsimd.iota(iota32[:], pattern=[[1, NCAND]], base=0,
                       channel_multiplier=0)
        iota32f = cst.tile([P, NCAND], f32)
        nc.vector.tensor_copy(out=iota32f[:], in_=iota32[:])

        th_b = cst.tile([P, 2], f32)
        nc.sync.dma_start(out=th_b[:], in_=io["theta"].broadcast_to([P, 2]))
        # sigmoid(theta) = 1 / (1 + exp(-theta)); DVE reciprocal for accuracy
        th_e = cst.tile([P, 2], f32)
        nc.scalar.activation(th_e[:], th_b[:], AF.Exp, scale=-1.0)
        nc.vector.tensor_scalar_add(th_e[:], th_e[:], 1.0)
        ts_sb = cst.tile([P, 2], f32)
        nc.vector.reciprocal(ts_sb[:], th_e[:])
        tsg = cst.tile([P, 2], f32)  # sigmoid(theta) * G
        nc.vector.tensor_scalar_mul(tsg[:], ts_sb[:], G)

        b_rep = cst.tile([P, DIN], f32)
        nc.sync.dma_start(out=b_rep[:], in_=io["bvec"].broadcast_to([P, DIN]))

        wt_sb = cst.tile([DIN, DIN], f32r)
        nc.sync.dma_start(out=wt_sb[:], in_=io["wt"])

        rowf_sb = cst.tile([P, NT], f32)
        nc.sync.dma_start(out=rowf_sb[:], in_=io["rowf"])

        # raw fp8 weights (tile+pair-interleaved A^T of the block): load now.
        # layout [P, NT, 2, P]: per partition k', slot (it, i) holds the
        # contiguous 128 block-columns of scaled A^T row (pair*256+i*128+k')
        w8 = []
        for pr in range(NPAIR):
            wtile = wpool.tile([P, NT, 2, P], f8, tag=f"w{pr}", name=f"w{pr}")
            nc.sync.dma_start(
                out=wtile[:],
                in_=io["a_t8"][pr * P:(pr + 1) * P, :].rearrange(
                    "p (nt two c) -> p nt two c", nt=NT, two=2))
            w8.append(wtile)

        # ---------------- block load (bf16) + degree d ----------------
        # rowsums split across Scalar (tiles 0,1 via activation accumulate)
        # and Vector (tiles 2,3) to shorten the AllGather critical path
        C = [cpool.tile([P, N], bf16, tag=f"c{it}", name=f"C{it}") for it in range(NT)]
        junk = cst.tile([P, N // 4], bf16)
        dm_sb = cst.tile([P, NT], f32)
        dmq = cst.tile([P, NT, 4], f32)
        for it in range(NT):
            quart = N // 4
            for qq in range(4):
                nc.sync.dma_start(
                    out=C[it][:, qq * quart:(qq + 1) * quart],
                    in_=io["a_blk"][it * P:(it + 1) * P, qq * quart:(qq + 1) * quart])
                if it < 2:
                    nc.scalar.activation(junk[:], C[it][:, qq * quart:(qq + 1) * quart],
                                         AF.Copy, accum_out=dmq[:, it, qq:qq + 1])
                else:
                    nc.vector.tensor_reduce(out=dmq[:, it, qq:qq + 1],
                                            in_=C[it][:, qq * quart:(qq + 1) * quart],
                                            axis=AX.X, op=OP.add)
            nc.vector.tensor_reduce(out=dm_sb[:, it:it + 1], in_=dmq[:, it, :],
                                    axis=AX.X, op=OP.add)
        # local per-row scale factors
        dinv2_blk = cst.tile([P, NT], f32)
        nc.vector.reciprocal(dinv2_blk[:], dm_sb[:])
        dinv_blk = cst.tile([P, NT], f32)
        nc.scalar.activation(dinv_blk[:], dinv2_blk[:], AF.Sqrt)
        if STAGE < 2:
            return
        # ---------------- AllGather #1 (d) ----------------
        g1_in = dram.tile([NT, P], f32)
        g1_out = dram.tile([LC, P], f32)
        with tc.tile_pool(name="psA", bufs=2, space="PSUM") as psA:
            dmT_ps = psA.tile([NT, P], f32)
            nc.tensor.transpose(out=dmT_ps[:], in_=dm_sb[:], identity=ident[:])
            dmT = cst.tile([NT, P], f32)
            nc.scalar.activation(dmT[:], dmT_ps[:], AF.Copy)
            nc.sync.dma_start(out=g1_in[:], in_=dmT[:])
            nc.gpsimd.collective_compute(
                "AllGather", OP.bypass,
                replica_groups=[list(range(NCORES))],
                ins=[g1_in.opt()], outs=[g1_out.opt()],
            )
            # (G*t0)-scale of the block overlaps the AllGather wait
            for it in range(NT):
                nc.vector.tensor_scalar_mul(C[it][:], C[it][:], tsg[:, 0:1])
            da_sb = cst.tile([LC, P], f32)
            nc.sync.dma_start(out=da_sb[:], in_=g1_out[:])

            # dinv (flat, global order, bf16) to DRAM for broadcast loads
            rda = cst.tile([LC, P], f32)
            nc.vector.reciprocal(rda[:], da_sb[:])
            dinv32 = cst.tile([LC, P], f32)
            nc.scalar.activation(dinv32[:], rda[:], AF.Sqrt)
            dinvb = cst.tile([LC, P], bf16)
            nc.vector.tensor_copy(out=dinvb[:], in_=dinv32[:])
            dinv_flat = dram.tile([1, N], bf16)
            nc.sync.dma_start(
                out=dinv_flat[:].rearrange("one (a b) -> (one a) b", a=LC),
                in_=dinvb[:])

            # W column scale = G*t1 / d  in [P, LC] layout (transpose of rda)
            wsc_ps = psA.tile([P, LC], f32)
            nc.tensor.transpose(out=wsc_ps[:], in_=rda[:], identity=ident[:LC, :LC])
            wscale = cst.tile([P, LC], f32)
            nc.scalar.activation(wscale[:], wsc_ps[:], AF.Copy, scale=tsg[:, 1:2])

        if STAGE < 3:
            return
        # scale raw fp8 weights in place: slot i of pair pr scaled by G*t1/d[lc]
        for pr in range(NPAIR):
            for i in range(2):
                lc = 2 * pr + i
                nc.scalar.activation(w8[pr][:, :, i, :], w8[pr][:, :, i, :],
                                     AF.Copy, scale=wscale[:, lc:lc + 1])

        if STAGE < 4:
            return
        # ---------------- main loop: B' = A_R @ diag(G t1/d) @ A ----------------
        # per-phase ranking candidates (proxy U = C' * dinv), merged in tail
        m8p = cst.tile([P, NT, NCAND], bf16)
        i8pf = cst.tile([P, NT, NCAND], f32)
        d2part = [cst.tile([P, PHASES], f32, tag=f"d2p{it}", name=f"d2p{it}")
                  for it in range(NT)]
        ciipart = [cst.tile([P, PHASES], f32, tag=f"cip{it}", name=f"cip{it}")
                   for it in range(NT)]
        mainctx = ExitStack()
        with mainctx:
            astream = mainctx.enter_context(tc.tile_pool(name="astream", bufs=12))
            dsp = mainctx.enter_context(tc.tile_pool(name="dstripe", bufs=2))
            scrp = mainctx.enter_context(tc.tile_pool(name="scr", bufs=2))
            i8scr = mainctx.enter_context(tc.tile_pool(name="i8scr", bufs=2))
            psM = mainctx.enter_context(tc.tile_pool(name="psM", bufs=8,
                                                     space="PSUM"))

            for q in range(PHASES):
                q0 = q * PW
                dstripe = dsp.tile([P, PW], bf16, tag="dstripe")
                nc.sync.dma_start(out=dstripe[:],
                                  in_=dinv_flat[:, q0:q0 + PW].broadcast_to([P, PW]))
                psums = [psM.tile([P, 512], f32, tag="acc", name=f"ps{q}_{i}")
                         for i in range(2 * NT)]
                for pr in range(NPAIR):
                    asl = astream.tile([P, 2, PW], f8, tag="astream")
                    # pair-interleaved phase-major layout: each partition line
                    # is 2KB-contiguous; split by partition halves over queues
                    for rq in range(2):
                        nc.sync.dma_start(
                            out=asl[rq * 64:(rq + 1) * 64, :, :],
                            in_=io["a_ph8"][(q * NPAIR + pr) * P + rq * 64:
                                            (q * NPAIR + pr) * P + (rq + 1) * 64,
                                            :].rearrange("p (two w) -> p two w",
                                                         two=2))
                    for it in range(NT):
                        lhs = w8[pr][:, it, :, :]
                        for jt in range(JT):
                            bank = it * 2 + jt // 2
                            half = jt % 2
                            nc.tensor.matmul(
                                out=psums[bank][:, half * 256:(half + 1) * 256],
                                lhsT=lhs,
                                rhs=asl[:, :, jt * 256:(jt + 1) * 256],
                                start=(pr == 0), stop=(pr == NPAIR - 1),
                                perf_mode=PM.DoubleRow)
                # free all PSUM banks first (PE of phase q+1 waits on these)
                for it in range(NT):
                    for jc in range(2):
                        cs = C[it][:, q0 + jc * 512:q0 + (jc + 1) * 512]
                        nc.vector.tensor_add(out=cs, in0=psums[it * 2 + jc][:],
                                             in1=cs)
                # d2 row-sum partials + per-phase top-8 candidates + diag pick
                i8q = i8scr.tile([P, NT, 8], u32, tag="i8q")
                for it in range(NT):
                    stripe = C[it][:, q0:q0 + PW]
                    scr1 = scrp.tile([P, PW], bf16, tag="scr")
                    nc.vector.tensor_mul(scr1[:], stripe, dstripe[:])
                    nc.vector.tensor_reduce(out=d2part[it][:, q:q + 1],
                                            in_=scr1[:], axis=AX.X, op=OP.add)
                    nc.vector.max(out=m8p[:, it, q * 8:(q + 1) * 8], in_=scr1[:])
                    nc.vector.max_index(out=i8q[:, it, :],
                                        in_max=m8p[:, it, q * 8:(q + 1) * 8],
                                        in_values=scr1[:])
                    scr2 = scrp.tile([P, PW], bf16, tag="scr")
                    nc.vector.tensor_scalar(out=scr2[:],
                                            in0=col_iota[:, q0:q0 + PW],
                                            scalar1=rowf_sb[:, it:it + 1],
                                            scalar2=None, op0=OP.is_equal)
                    nc.vector.tensor_mul(scr2[:], scr2[:], stripe)
                    nc.vector.tensor_reduce(out=ciipart[it][:, q:q + 1],
                                            in_=scr2[:], axis=AX.X, op=OP.add)
                # phase-local indices -> global column indices (f32)
                nc.vector.tensor_copy(out=i8pf[:, :, q * 8:(q + 1) * 8],
                                      in_=i8q[:])
                nc.vector.tensor_scalar_add(i8pf[:, :, q * 8:(q + 1) * 8],
                                            i8pf[:, :, q * 8:(q + 1) * 8],
                                            float(q0))

        if STAGE < 5:
            return
        # ---------------- d2 + AllGather #2 (emitted first: feeds the CC) ----
        tailctx = ExitStack()
        with tailctx:
            tp = tailctx.enter_context(tc.tile_pool(name="tail", bufs=1))
            tscr = tailctx.enter_context(tc.tile_pool(name="tscr", bufs=2))
            psT = tailctx.enter_context(tc.tile_pool(name="psT", bufs=4,
                                                     space="PSUM"))

            d2m = tp.tile([P, NT], f32)
            for it in range(NT):
                nc.vector.tensor_reduce(out=d2m[:, it:it + 1], in_=d2part[it][:],
                                        axis=AX.X, op=OP.add)
            # d2 = 1 + dinv * sum / G
            nc.vector.tensor_mul(d2m[:], d2m[:], dinv_blk[:])
            nc.vector.tensor_scalar(out=d2m[:], in0=d2m[:], scalar1=1.0 / G,
                                    scalar2=1.0, op0=OP.mult, op1=OP.add)

            dinv2b2 = tp.tile([P, NT], f32)  # dinv2b^2 = 1/d2
            nc.vector.reciprocal(dinv2b2[:], d2m[:])
            dinv2b = tp.tile([P, NT], f32)
            nc.scalar.activation(dinv2b[:], dinv2b2[:], AF.Sqrt)

            g2_in = dram.tile([NT, P], f32)
            g2_out = dram.tile([LC, P], f32)
            d2T_ps = psT.tile([NT, P], f32, tag="tr")
            nc.tensor.transpose(out=d2T_ps[:], in_=d2m[:], identity=ident[:])
            d2T = tp.tile([NT, P], f32)
            nc.scalar.activation(d2T[:], d2T_ps[:], AF.Copy)
            nc.sync.dma_start(out=g2_in[:], in_=d2T[:])
            nc.gpsimd.collective_compute(
                "AllGather", OP.bypass,
                replica_groups=[list(range(NCORES))],
                ins=[g2_in.opt()], outs=[g2_out.opt()],
            )

            # ---- AG2-independent work (overlaps the collective) ----
            cii = tp.tile([P, NT], f32)
            for it in range(NT):
                nc.vector.tensor_reduce(out=cii[:, it:it + 1], in_=ciipart[it][:],
                                        axis=AX.X, op=OP.add)

            # merge the 32 candidates per row tile: top-8 of U + global index
            m8b = tp.tile([P, NT, 8], bf16)
            p8 = tp.tile([P, NT, 8], u32)
            for it in range(NT):
                nc.vector.max(out=m8b[:, it, :], in_=m8p[:, it, :])
                nc.vector.max_index(out=p8[:, it, :], in_max=m8b[:, it, :],
                                    in_values=m8p[:, it, :])
            m8all = tp.tile([P, NT, 8], f32)
            nc.vector.tensor_copy(out=m8all[:], in_=m8b[:])
            p8f = tp.tile([P, NT, 8], f32)
            nc.vector.tensor_copy(out=p8f[:], in_=p8[:])
            # decode merge positions (0..31) to global indices via one-hot
            match = tp.tile([P, NT, 8, NCAND], bf16)
            nc.vector.tensor_tensor(
                out=match[:],
                in0=p8f[:].unsqueeze(3).to_broadcast([P, NT, 8, NCAND]),
                in1=iota32f[:].unsqueeze(1).unsqueeze(2).to_broadcast(
                    [P, NT, 8, NCAND]),
                op=OP.is_equal)
            msel = tp.tile([P, NT, 8, NCAND], f32)
            nc.vector.tensor_tensor(
                out=msel[:], in0=match[:],
                in1=i8pf[:].unsqueeze(2).to_broadcast([P, NT, 8, NCAND]),
                op=OP.mult)
            i8f = tp.tile([P, NT, 8], f32)
            nc.vector.tensor_reduce(out=i8f[:], in_=msel[:], axis=AX.X, op=OP.add)

            # rowf replicated along the 8-wide groups
            kill = tp.tile([P, NT, 8], f32)
            nc.vector.tensor_tensor(out=kill[:], in0=i8f[:],
                                    in1=rowf_sb[:].unsqueeze(2).to_broadcast([P, NT, 8]),
                                    op=OP.is_equal)
            # delete diag entry from candidates
            m8k = tp.tile([P, NT, 8], f32)
            nc.vector.tensor_scalar(out=m8k[:], in0=kill[:], scalar1=BIGNEG,
                                    scalar2=None, op0=OP.mult)
            nc.vector.tensor_sub(m8k[:], m8all[:], m8k[:])
            # diag position within each group (99 if absent)
            posw = tp.tile([P, NT, 8], f32)
            nc.vector.tensor_tensor(out=posw[:], in0=kill[:],
                                    in1=iota8m[:].unsqueeze(1).to_broadcast([P, NT, 8]),
                                    op=OP.mult)
            pos = tp.tile([P, NT], f32)
            nc.vector.tensor_reduce(out=pos[:], in_=posw[:], axis=AX.X, op=OP.add)
            nc.vector.tensor_scalar_add(pos[:], pos[:], 99.0)
            shift = tp.tile([P, NT, 8], f32)
            nc.vector.tensor_tensor(out=shift[:],
                                    in0=iota8f[:].unsqueeze(1).to_broadcast([P, NT, 8]),
                                    in1=pos[:].unsqueeze(2).to_broadcast([P, NT, 8]),
                                    op=OP.is_ge)
            # top-(k-1) values/indices skipping the diag slot
            val = tp.tile([P, NT, topn], f32)
            nc.vector.tensor_sub(val[:], m8k[:, :, 1:1 + topn], m8k[:, :, 0:topn])
            nc.vector.tensor_mul(val[:], val[:], shift[:, :, 0:topn])
            nc.vector.tensor_add(val[:], val[:], m8k[:, :, 0:topn])
            idxf = tp.tile([P, NT, topn], f32)
            nc.vector.tensor_sub(idxf[:], i8f[:, :, 1:1 + topn], i8f[:, :, 0:topn])
            nc.vector.tensor_mul(idxf[:], idxf[:], shift[:, :, 0:topn])
            nc.vector.tensor_add(idxf[:], idxf[:], i8f[:, :, 0:topn])
            idxu = tp.tile([P, NT, topn], u32)
            nc.vector.tensor_copy(out=idxu[:], in_=idxf[:])

            # output coefficients (AG2-independent parts)
            c_off = tp.tile([P, NT], f32)
            nc.vector.tensor_mul(c_off[:], dinv2b[:], dinv_blk[:])
            nc.vector.tensor_scalar_mul(c_off[:], c_off[:], 1.0 / G)
            c_diag = tp.tile([P, NT], f32)
            nc.vector.tensor_mul(c_diag[:], dinv2_blk[:], cii[:])
            nc.vector.tensor_scalar(out=c_diag[:], in0=c_diag[:], scalar1=1.0 / G,
                                    scalar2=1.0, op0=OP.mult, op1=OP.add)
            nc.vector.tensor_mul(c_diag[:], c_diag[:], dinv2b2[:])
            cval = tp.tile([P, NT, topn], f32)
            nc.vector.tensor_tensor(out=cval[:], in0=val[:],
                                    in1=c_off[:].unsqueeze(2).to_broadcast([P, NT, topn]),
                                    op=OP.mult)

            # gathers (x rows): one batched indirect DMA per tile + one
            # strided direct DMA for the diagonal rows (own block of x)
            xg = tp.tile([P, NT, (topn + 1), DIN], f32)
            for it in range(NT):
                nc.gpsimd.indirect_dma_start(
                    out=xg[:, it, 0:topn, :], out_offset=None,
                    in_=io["x"],
                    in_offset=bass.IndirectOffsetOnAxis(ap=idxu[:, it, :],
                                                        axis=0))
            nc.sync.dma_start(
                out=xg[:, :, topn, :],
                in_=io["xblk"].rearrange("(nt pp) d -> pp nt d", nt=NT))

            if STAGE < 6:
                return
            # ---- AG2-dependent: dinv2b over all columns, gathered per pick ----
            d2a_sb = tp.tile([LC, P], f32)
            nc.sync.dma_start(out=d2a_sb[:], in_=g2_out[:])
            v2 = tp.tile([LC, P], f32)
            nc.vector.reciprocal(v2[:], d2a_sb[:])
            nc.scalar.activation(v2[:], v2[:], AF.Sqrt)
            d2bflat = dram.tile([N, 1], f32)
            nc.sync.dma_start(
                out=d2bflat[:].rearrange("(a b) one -> a (b one)", a=LC),
                in_=v2[:])
            g2b = tp.tile([P, NT, topn], f32)
            nc.gpsimd.indirect_dma_start(
                out=g2b[:], out_offset=None,
                in_=d2bflat[:],
                in_offset=bass.IndirectOffsetOnAxis(ap=idxu[:, :, :], axis=0))
            nc.vector.tensor_mul(cval[:], cval[:], g2b[:])

            # z = c_diag * x[i] + sum_t cval_t * x[j_t]   (batched over tiles)
            zall = tp.tile([P, NT, DIN], f32)
            nc.vector.tensor_tensor(out=zall[:], in0=xg[:, :, topn, :],
                                    in1=c_diag[:].unsqueeze(2).to_broadcast([P, NT, DIN]),
                                    op=OP.mult)
            zt = tp.tile([P, NT, DIN], f32)
            for t in range(topn):
                nc.vector.tensor_tensor(out=zt[:], in0=xg[:, :, t, :],
                                        in1=cval[:, :, t:t + 1].to_broadcast([P, NT, DIN]),
                                        op=OP.mult)
                nc.vector.tensor_add(zall[:], zall[:], zt[:])

            # out = z @ W.T + b  (per tile on PE)
            for it in range(NT):
                zT_ps = psT.tile([DIN, P], f32, tag="tr")
                nc.tensor.transpose(out=zT_ps[:], in_=zall[:, it, :],
                                    identity=ident[:])
                zT = tscr.tile([DIN, P], f32, tag="zT")
                nc.scalar.activation(zT[:].bitcast(f32r), zT_ps[:], AF.Copy)
                o_ps = psT.tile([P, DIN], f32, tag="ops")
                nc.tensor.matmul(out=o_ps[:], lhsT=zT[:].bitcast(f32r),
                                 rhs=wt_sb[:], start=True, stop=True)
                o_sb = tscr.tile([P, DIN], f32, tag="osb")
                nc.vector.tensor_add(o_sb[:], o_ps[:], b_rep[:])
                nc.sync.dma_start(out=io["out_blk"][it * P:(it + 1) * P, :],
                                  in_=o_sb[:])


def _build(N, NB, DIN, k):
    key = (N, NB, DIN, k, os.environ.get("K_STAGE", "9"))
    if key in _PROGRAM_CACHE:
        return _PROGRAM_CACHE[key]
    NT, PW, PHASES, NPAIR, JT, LC = _geom(N, NB)
    nc = bacc.Bacc("TRN2", target_bir_lowering=False, debug=False,
                   num_devices=NCORES)
    io = {
        "a_ph8": nc.dram_tensor("a_ph8", [PHASES * NPAIR * P, 2 * PW], f8,
                                kind="ExternalInput").ap(),
        "a_blk": nc.dram_tensor("a_blk", [NB, N], bf16,
                                kind="ExternalInput").ap(),
        "a_t8": nc.dram_tensor("a_t8", [NPAIR * P, NT * 2 * P], f8,
                               kind="ExternalInput").ap(),
        "x": nc.dram_tensor("x", [N, DIN], f32, kind="ExternalInput").ap(),
        "xblk": nc.dram_tensor("xblk", [NB, DIN], f32,
                               kind="ExternalInput").ap(),
        "wt": nc.dram_tensor("wt", [DIN, DIN], f32r, kind="ExternalInput").ap(),
        "bvec": nc.dram_tensor("bvec", [1, DIN], f32, kind="ExternalInput").ap(),
        "theta": nc.dram_tensor("theta", [1, 2], f32, kind="ExternalInput").ap(),
        "rowf": nc.dram_tensor("rowf", [P, NT], f32, kind="ExternalInput").ap(),
        "out_blk": nc.dram_tensor("out_blk", [NB, DIN], f32,
                                  kind="ExternalOutput").ap(),
    }
    with tile.TileContext(nc) as tc:
        _emit(tc, io, N, NB, DIN, k)
    nc.compile()
    _PROGRAM_CACHE[key] = nc
    return nc


def make_in_maps(x, A, theta, W, b, k, N, NB, DIN):
    A = np.ascontiguousarray(np.asarray(A, np.float32))
    x = np.ascontiguousarray(np.asarray(x, np.float32))
    theta = np.ascontiguousarray(np.asarray(theta, np.float32)).reshape(1, 2)
    W = np.asarray(W, np.float32)
    b = np.ascontiguousarray(np.asarray(b, np.float32)).reshape(1, DIN)
    wt = np.ascontiguousarray(W.T)
    NT, PW, PHASES, NPAIR, JT, LC = _geom(N, NB)
    F8 = mybir.dt.np(mybir.dt.float8e4)
    BF = mybir.dt.np(mybir.dt.bfloat16)
    A8 = A.astype(F8)
    # stream layout: row ((q*NPAIR + pair)*128 + p) holds A rows
    # (pair*256 + p) and (pair*256 + 128 + p), columns q*PW..(q+1)*PW
    a_ph8 = np.ascontiguousarray(
        A8.reshape(NPAIR, 2, P, PHASES, PW)
          .transpose(3, 0, 2, 1, 4)
          .reshape(PHASES * NPAIR * P, 2 * PW))
    in_maps = []
    for m in range(NCORES):
        rows = slice(m * NB, (m + 1) * NB)
        a_blk = np.ascontiguousarray(A[rows].astype(BF))
        # weights row (pair*128 + p), slot (it, i):
        #   A^T[pair*256 + i*128 + p, it*128:(it+1)*128] of the block
        a_t8 = np.ascontiguousarray(
            A8[rows].T.reshape(NPAIR, 2, P, NT, P)
                      .transpose(0, 2, 3, 1, 4)
                      .reshape(NPAIR * P, NT * 2 * P))
        ridx = (m * NB + np.arange(NB)).reshape(NT, P).T  # [P, NT]
        in_maps.append({
            "a_ph8": a_ph8,
            "a_blk": a_blk,
            "a_t8": a_t8,
            "x": x,
            "xblk": np.ascontiguousarray(x[rows]),
            "wt": wt,
            "bvec": b,
            "theta": theta,
            "rowf": np.ascontiguousarray(ridx.astype(np.float32)),
        })
    return in_maps


def kernel(x, A, theta, W, b, k, **extra):
    k = int(k)
    assert 1 <= k <= 8, f"k={k} unsupported"
    N = int(A.shape[0])
    DIN = int(x.shape[1])
    NB = N // NCORES
    nc = _build(N, NB, DIN, k)
    in_maps = make_in_maps(x, A, theta, W, b, k, N, NB, DIN)
    trace = bool(int(os.environ.get("BASS_KERNEL_TRACE", "0")))
    t0 = time.monotonic()
    res = bass_utils.run_bass_kernel_spmd(
        nc, in_maps, core_ids=list(range(NCORES)), trace=trace)
    t1 = time.monotonic()
    LAST_RUN_INFO.clear()
    LAST_RUN_INFO.update({
        "wall_s": t1 - t0,
        "exec_time_ns": res.exec_time_ns,
        "profile_json": res.profile_json,
    })
    out = np.concatenate([res.results[m]["out_blk"] for m in range(NCORES)], axis=0)
    return out.astype(np.float32)
